# revision 6
# baseline (speedup 1.0000x reference)
"""DILATE divergence loss on 8 Trainium2 NeuronCores, data-parallel over batch.

Forward-only formulation (no backward pass, no per-row DRAM traffic):
  scaled units X' = X/gamma.
  O-DP (hard min):  O[i,j] = dsq[i,j] + min(O[i-1,j-1], O[i-1,j], O[i,j-1])
     one scan/row:  state = min(pm[j], state) + dsq[j],  pm = min(O'_s, O')
  z-correction:     z[i,j] = c_j z[i,j-1] + a_j z[i-1,j-1] + b_j z[i-1,j]
     a = e^{m-O'_s}, b = e^{m-O'}, c = e^{m-O_s},  m = O - dsq  (3-way-min stabilizer)
     soft-DTW:      R' = O - ln z   (exact)
  temporal loss via forward-mode JVP (exact, replaces the whole backward):
     T_b = <dR/dD, Omega> with Omega[i,j] = (i-j)^2.  zeta := z * Rdot satisfies
     zeta[i,j] = c_j zeta[i,j-1] + a_j zeta' _{j-1} + b_j zeta'_j + z[i,j]*Omega[i,j]
     (same a,b,c weights).  T_b = zeta_end / z_end.

Partition layout per core (lockstep [40,*] ops - free-dim bound, partitions free):
  0-7 xy, 8-15 xx, 16-23 yy (z-group), 24-31 zeroed dummies,
  32-39 duplicate xy DP running ONE ROW BEHIND (zeta group): its scan rows hold
  zeta, and the z*Omega source reads the z-group's previous-row z (partition
  starts 0 and 32 are both legal in one instruction).

loss = alpha * mean_b gamma*(R'_xy - (R'_xx+R'_yy)/2) + (1-alpha) * mean_b(T_b)/N^2
"""

import numpy as np

ALPHA = 0.5
GAMMA = 0.01
B, N, DF = 64, 256, 1
NCORES = 8
BPC = B // NCORES          # 8 batches per core
PZ = 3 * BPC               # 24 z-problems
PT = 40                    # partitions incl. zeta group at 32-39
INF = 1.0e9
SQG = float(np.sqrt(GAMMA))
CW = N + (N + 1) + (2 * N - 1)   # ss | negts | wsq  = 1024

_PROGRAM_CACHE = {}


def build_program():
    import concourse.mybir as mybir
    from concourse import bacc
    from concourse.tile import TileContext

    dt = mybir.dt.float32
    Alu = mybir.AluOpType
    Act = mybir.ActivationFunctionType

    nc = bacc.Bacc("TRN2", target_bir_lowering=False, debug=False, num_devices=NCORES)

    cpk_d = nc.dram_tensor("cpack", [PT, CW], dt, kind="ExternalInput").ap()
    fin_d = nc.dram_tensor("finals", [PZ, 1], dt, kind="ExternalOutput").ap()
    zz_d = nc.dram_tensor("zzout", [BPC, 2], dt, kind="ExternalOutput").ap()

    V = nc.vector
    G = nc.gpsimd
    S = nc.scalar

    with TileContext(nc) as tc:
        with (
            tc.tile_pool(name="state", bufs=1) as st,
            tc.tile_pool(name="work", bufs=4) as wk,
        ):
            cpk = st.tile([PT, CW], dt, name="cpk")
            nc.sync.dma_start(cpk[:], cpk_d[:])
            ss = cpk[:, 0:N]
            negts = cpk[:, N:2 * N + 1]           # [PT, N+1]
            wsq = cpk[:, 2 * N + 1:4 * N]         # [PT, 2N-1]

            obufs = [st.tile([PT, N + 1], dt, name=f"ob{k}") for k in range(3)]
            zbufs = [st.tile([PT, N + 1], dt, name=f"zb{k}") for k in range(3)]

            # row -1 inits (slot index -1 -> bufs[2])
            G.memset(obufs[2][:], INF)
            G.memset(obufs[2][:, 0:1], 0.0)        # corner O[-1,-1] = 0
            G.memset(obufs[0][:, 0:1], INF)        # left pads of row buffers
            G.memset(obufs[1][:, 0:1], INF)
            G.memset(zbufs[2][:], 0.0)
            G.memset(zbufs[2][0:PZ, 0:1], 1.0)     # corner z = 1 (zeta corner stays 0)
            G.memset(zbufs[0][:, 0:1], 0.0)
            G.memset(zbufs[1][:, 0:1], 0.0)

            dsqs = {}
            e3s = {}

            def stage_B(i):
                """O-chain for step i: dsq, pm, scanO -> OROW(i)."""
                Op = obufs[(i - 1) % 3]
                O = obufs[i % 3]
                dsq = wk.tile([PT, N], dt, tag="dsq")
                dsqs[i] = dsq
                S.activation(dsq[:], ss, Act.Square, bias=negts[:, i:i + 1])
                pm = wk.tile([PT, N], dt, tag="pm")
                V.tensor_tensor(pm[:], Op[:, 0:N], Op[:, 1:N + 1], Alu.min)
                V.tensor_tensor_scan(O[:, 1:N + 1], pm[:], dsq[:], INF,
                                     Alu.min, Alu.add)

            def stage_A(i):
                """weights for step i: m, A3, E3 (needs OROW(i), OROW(i-1))."""
                Op = obufs[(i - 1) % 3]
                O = obufs[i % 3]
                dsq = dsqs.pop(i)
                m = wk.tile([PT, N], dt, tag="m")
                V.tensor_tensor(m[:], O[:, 1:N + 1], dsq[:], Alu.subtract)
                A3 = wk.tile([PT, 3 * N], dt, tag="A3")
                V.tensor_tensor(A3[:, 0:N], m[:], Op[:, 0:N], Alu.subtract)
                V.tensor_tensor(A3[:, N:2 * N], m[:], Op[:, 1:N + 1], Alu.subtract)
                V.tensor_tensor(A3[:, 2 * N:3 * N], m[:], O[:, 0:N], Alu.subtract)
                E3 = wk.tile([PT, 3 * N], dt, tag="E3")
                e3s[i] = E3
                S.activation(E3[:], A3[:], Act.Exp)

            def stage_C(i):
                """z/zeta scan for step i (needs E3(i), ZROW(i-1))."""
                E3 = e3s.pop(i)
                zp = zbufs[(i - 1) % 3]
                z = zbufs[i % 3]
                p1 = wk.tile([PT, N], dt, tag="p1")
                V.tensor_tensor(p1[:], E3[:, 0:N], zp[:, 0:N], Alu.mult)
                p2 = wk.tile([PT, N], dt, tag="p2")
                V.tensor_tensor(p2[:], E3[:, N:2 * N], zp[:, 1:N + 1], Alu.mult)
                prep = wk.tile([PT, N], dt, tag="prep")
                V.tensor_tensor(prep[:], p1[:], p2[:], Alu.add)
                if i > 0:
                    # zeta source: z-row (i-1) of xy  *  (r-j)^2 window, r = i-1
                    # (both inputs at base partition 0 - HW requires equal input
                    # bases; only the output lands on the zeta group at 32)
                    srct = wk.tile([PT, N], dt, tag="srct")
                    G.tensor_tensor(srct[32:40, :], zp[0:BPC, 1:N + 1],
                                    wsq[0:BPC, N - i:2 * N - i], Alu.mult)
                    V.tensor_tensor(prep[32:40, :], prep[32:40, :],
                                    srct[32:40, :], Alu.add)
                V.tensor_tensor_scan(z[:, 1:N + 1], E3[:, 2 * N:3 * N], prep[:],
                                     0.0, Alu.mult, Alu.add)

            # ---- software-pipelined emission: A runs 2 steps ahead of C ----
            stage_B(0)
            stage_A(0)
            # zeta-group row "-1" fixup: overwrite its garbage OROW(0) with
            # inits AFTER A(0) read it (garbage-DP args are <= 0, so A(0) rows
            # 32-39 stay finite; post-fixup they would hit exp(+INF)).
            G.memset(obufs[0][32:40, :], INF)
            G.memset(obufs[0][32:40, 0:1], 0.0)
            # restore left pad of the init slot for when it becomes OROW(2)
            G.memset(obufs[2][:, 0:1], INF)
            stage_B(1)
            stage_A(1)
            # restore left pad of the zeta-corner once A(1)/B(1) consumed it
            G.memset(obufs[0][32:40, 0:1], INF)

            for i in range(N + 1):
                stage_C(i)
                if i == 0:
                    G.memset(zbufs[2][:, 0:1], 0.0)  # left pad once init consumed
                if i + 2 <= N:
                    stage_B(i + 2)
                    stage_A(i + 2)
                if i == N - 1:
                    zN = zbufs[(N - 1) % 3]
                    ON = obufs[(N - 1) % 3]
                    lnz = wk.tile([PZ, 1], dt, tag="lnz")
                    S.activation(lnz[:], zN[0:PZ, N:N + 1], Act.Ln)
                    fin = wk.tile([PZ, 1], dt, tag="fin")
                    V.tensor_tensor(fin[:], ON[0:PZ, N:N + 1], lnz[:],
                                    Alu.subtract)
                    nc.sync.dma_start(fin_d[:], fin[:])
                    nc.sync.dma_start(zz_d[:, 0:1], zN[0:BPC, N:N + 1])
            zFin = zbufs[N % 3]
            nc.sync.dma_start(zz_d[:, 1:2], zFin[32:40, N:N + 1])

    nc.finalize()
    return nc


def get_program():
    if "nc" not in _PROGRAM_CACHE:
        _PROGRAM_CACHE["nc"] = build_program()
    return _PROGRAM_CACHE["nc"]


def make_in_maps(input, target):
    x = np.asarray(input, np.float32).reshape(B, N) / SQG
    y = np.asarray(target, np.float32).reshape(B, N) / SQG
    wsqv = ((np.arange(2 * N - 1) - (N - 1)).astype(np.float32) ** 2)
    in_maps = []
    for c in range(NCORES):
        sl = slice(c * BPC, (c + 1) * BPC)
        xs, ys = x[sl], y[sl]
        cpk = np.zeros((PT, CW), np.float32)
        # ss: col sequences (s): xy->x, xx->y, yy->x, zeta(xy)->x
        cpk[0:BPC, 0:N] = xs
        cpk[BPC:2 * BPC, 0:N] = ys
        cpk[2 * BPC:3 * BPC, 0:N] = xs
        cpk[32:40, 0:N] = xs
        # negts [PT, N+1]: -t_i at col i; zeta group lags one row
        cpk[0:BPC, N:2 * N] = -ys
        cpk[BPC:2 * BPC, N:2 * N] = -ys
        cpk[2 * BPC:3 * BPC, N:2 * N] = -xs
        cpk[32:40, N + 1:2 * N + 1] = -ys
        # wsq on xy rows (read at base partition 0) and zeta rows
        cpk[0:BPC, 2 * N + 1:4 * N] = wsqv[None, :]
        cpk[32:40, 2 * N + 1:4 * N] = wsqv[None, :]
        in_maps.append({"cpack": cpk})
    return in_maps


def combine_outputs(results):
    shape_terms = []
    t_terms = []
    for r in results:
        fin = np.asarray(r["finals"], np.float64).reshape(PZ)
        xy, xx, yy = fin[0:BPC], fin[BPC:2 * BPC], fin[2 * BPC:3 * BPC]
        shape_terms.append(GAMMA * (xy - 0.5 * (xx + yy)))
        zz = np.asarray(r["zzout"], np.float64).reshape(BPC, 2)
        t_terms.append(zz[:, 1] / zz[:, 0])
    loss_shape = float(np.mean(np.concatenate(shape_terms)))
    loss_temporal = float(np.mean(np.concatenate(t_terms))) / (N * N)
    return np.float32(ALPHA * loss_shape + (1.0 - ALPHA) * loss_temporal)


def kernel(input, target):
    from concourse import bass_utils
    nc = get_program()
    in_maps = make_in_maps(input, target)
    res = bass_utils.run_bass_kernel_spmd(nc, in_maps, core_ids=list(range(NCORES)))
    return combine_outputs(res.results)


if __name__ == "__main__":
    rng = np.random.default_rng(0)
    inp = rng.standard_normal((B, N, DF)).astype(np.float32)
    tgt = rng.standard_normal((B, N, DF)).astype(np.float32)
    print("loss:", kernel(input=inp, target=tgt))
